# revision 1
# baseline (speedup 1.0000x reference)
"""Trainium2 Bass kernel for windowed multi-head attention (Swin-style).

Problem: B=4096 windows x N=64 tokens x C=128 channels, H=4 heads, hd=32.
  qkv = x @ w_qkv ; attn = softmax(q k^T / sqrt(hd) + rel_bias) ; out = (attn v) @ w_proj + b

Sharding: data-parallel over windows, 512 windows per core on 8 cores.

Per-core dataflow (bf16 compute, fp32 PSUM):
  x --PE transpose--> xT [c, tok]
  qT = wq^T xT ; kT = wk^T xT     (weight-stationary; head h lives on partitions 32h)
  v  = x @ wv                     (xT-stationary, duplicated into both partition halves)
  attn^T[m,(h-even/odd rows)] = k q^T per (window, head)  via tile_position packing
  P = exp(attn^T) * exp(bias)^T   (ACT exp, DVE multiply: bias folded multiplicatively)
  s = ones^T @ P (PE), r ~ 1/s (DVE approx recip), Pn = P * r
  av^T[(h,d), n]                  (col-group packed -> [c, tok] layout)
  y = av @ w_proj + b             (avT-stationary; bias fused into DVE evacuation)

PSUM-bank rule respected throughout: no two in-flight matmuls with different
tile_position row-groups and the same column-group may target the same PSUM
bank (HW drain-port conflict, aborts execution).
"""

import sys

sys.path.insert(0, "/opt/trn_rl_repo")

import numpy as np
import ml_dtypes

WS = 8
H = 4
DIM = 128
N = WS * WS  # 64 tokens per window
HD = DIM // H  # 32
B = 4096
NCORES = 8
BC = B // NCORES  # 512 windows per core
ROWS = BC * N  # 32768 rows per core

SC_W = 4  # windows per superchunk
SC_ROWS = SC_W * N  # 256
N_SC = BC // SC_W  # 128 superchunks
GROUP = 4  # superchunks per DMA group
N_G = N_SC // GROUP  # 32 groups

bf16 = ml_dtypes.bfloat16


def _rel_pos_index(ws: int) -> np.ndarray:
    coords = np.stack(np.meshgrid(np.arange(ws), np.arange(ws), indexing="ij"))
    flat = coords.reshape(2, -1)
    rel = flat[:, :, None] - flat[:, None, :]
    rel = rel.transpose(1, 2, 0).astype(np.int64)
    rel[..., 0] += ws - 1
    rel[..., 1] += ws - 1
    rel[..., 0] *= 2 * ws - 1
    return rel.sum(-1)


_BUILT = {}


def _build_program(n_groups=N_G, compile=True, stage=6):
    """Build + compile the single-core Bass program (same program runs SPMD on all cores).

    stage < 6 truncates the per-superchunk pipeline for HW bisection and writes
    that stage's intermediate into y instead.
    """
    key = (n_groups, compile, stage)
    if key in _BUILT:
        return _BUILT[key]
    rows = n_groups * GROUP * SC_ROWS

    from contextlib import ExitStack

    import concourse.tile as tile
    from concourse import bacc, mybir
    from concourse.masks import make_identity

    f32 = mybir.dt.float32
    bf = mybir.dt.bfloat16
    EXP = mybir.ActivationFunctionType.Exp

    nc = bacc.Bacc("TRN2", target_bir_lowering=False, debug=False, enable_asserts=False)

    x_d = nc.dram_tensor("x", [rows, DIM], f32, kind="ExternalInput").ap()
    wq_d = nc.dram_tensor("wq_bf", [DIM, DIM], bf, kind="ExternalInput").ap()
    wk_d = nc.dram_tensor("wk_bf", [DIM, DIM], bf, kind="ExternalInput").ap()
    wv_d = nc.dram_tensor("wv_bf", [DIM, DIM], bf, kind="ExternalInput").ap()
    wp_d = nc.dram_tensor("wp_bf", [DIM, DIM], bf, kind="ExternalInput").ap()
    # eb[p, 256*b + 64*w + n] = exp(rel_bias[2b + (p>=64), n, p%64]), window-tiled
    eb_d = nc.dram_tensor("expbias_bf", [DIM, 2 * H * N], bf, kind="ExternalInput").ap()
    br_d = nc.dram_tensor("b_rep", [DIM, 2 * DIM], f32, kind="ExternalInput").ap()
    y_d = nc.dram_tensor("y", [rows, DIM], f32, kind="ExternalOutput").ap()

    with tile.TileContext(nc) as tc, ExitStack() as ctx:
        consts = ctx.enter_context(tc.tile_pool(name="consts", bufs=1))
        xp = ctx.enter_context(tc.tile_pool(name="xp", bufs=2))
        sb1 = ctx.enter_context(tc.tile_pool(name="sb1", bufs=3))
        sb2 = ctx.enter_context(tc.tile_pool(name="sb2", bufs=3))
        yp = ctx.enter_context(tc.tile_pool(name="yp", bufs=2))

        ps_xq = ctx.enter_context(tc.tile_pool(name="ps_xq", bufs=1, space="PSUM"))
        ps_v = ctx.enter_context(tc.tile_pool(name="ps_v", bufs=1, space="PSUM"))
        ps_at = ctx.enter_context(tc.tile_pool(name="ps_at", bufs=1, space="PSUM"))
        ps_sm = ctx.enter_context(tc.tile_pool(name="ps_sm", bufs=1, space="PSUM"))
        ps_av = ctx.enter_context(tc.tile_pool(name="ps_av", bufs=1, space="PSUM"))
        ps_y = ctx.enter_context(tc.tile_pool(name="ps_y", bufs=1, space="PSUM"))

        # constants
        wq = consts.tile([DIM, DIM], bf)
        wk = consts.tile([DIM, DIM], bf)
        wv = consts.tile([DIM, DIM], bf)
        wp = consts.tile([DIM, DIM], bf)
        eb = consts.tile([DIM, 2 * H * N], bf)
        br = consts.tile([DIM, 2 * DIM], f32)
        nc.sync.dma_start(wq[:], wq_d)
        nc.sync.dma_start(wk[:], wk_d)
        nc.sync.dma_start(wv[:], wv_d)
        nc.sync.dma_start(wp[:], wp_d)
        nc.sync.dma_start(eb[:], eb_d)
        nc.sync.dma_start(br[:], br_d)

        ident = consts.tile([DIM, DIM], f32)
        make_identity(nc, ident[:])
        ones64 = consts.tile([DIM, N], bf)
        nc.vector.memset(ones64[:], 1.0)

        for g in range(n_groups):
            g_r0 = g * GROUP * SC_ROWS
            x_g = xp.tile([128, 2 * GROUP, DIM], f32, tag="x_g")
            nc.sync.dma_start(
                out=x_g[:],
                in_=x_d[g_r0 : g_r0 + GROUP * SC_ROWS, :].rearrange(
                    "(t p) c -> p t c", p=128
                ),
            )
            y_g = yp.tile([128, 2 * GROUP, DIM], f32, tag="y_g")

            for s in range(GROUP):
                # ---- transpose x -> xT [c, tok]; qT, kT (weight stationary) ----
                # ps_xq [128, 768] f32: bank0 = xt(0:256) + q(256:512); bank1 = k(512:768)
                xq_ps = ps_xq.tile([128, 3 * SC_ROWS], f32, tag="xq_ps")
                for t in range(2):
                    nc.tensor.matmul(
                        xq_ps[:, t * 128 : (t + 1) * 128],
                        x_g[:, 2 * s + t, :],
                        ident[:],
                        is_transpose=True,
                        start=True,
                        stop=True,
                    )
                xt = sb1.tile([128, SC_ROWS], bf, tag="xt")
                nc.scalar.copy(xt[:], xq_ps[:, 0:SC_ROWS])

                if stage < 2:
                    nc.vector.tensor_copy(
                        y_g[:, 2 * s : 2 * s + 2, :].rearrange("p a b -> p (a b)"),
                        xt[:],
                    )
                    continue

                nc.tensor.matmul(
                    xq_ps[:, SC_ROWS : 2 * SC_ROWS], wq[:], xt[:], start=True, stop=True
                )
                nc.tensor.matmul(
                    xq_ps[:, 2 * SC_ROWS : 3 * SC_ROWS], wk[:], xt[:], start=True, stop=True
                )
                qk = sb1.tile([128, 2 * SC_ROWS], bf, tag="qk")
                nc.scalar.copy(qk[:], xq_ps[:, SC_ROWS : 3 * SC_ROWS])

                # ---- v = x @ wv, duplicated into both partition halves ----
                # ps_v [128, 512] f32: window w -> cols 128w; rows 0:64 and 64:128 same
                v_ps = ps_v.tile([128, 4 * DIM], f32, tag="v_ps")
                for w in range(SC_W):
                    for cp in range(2):
                        nc.tensor.matmul(
                            v_ps[64 * cp : 64 * cp + 64, 128 * w : 128 * (w + 1)],
                            xt[:, 64 * w : 64 * (w + 1)],
                            wv[:],
                            tile_position=(0, 64 * cp),
                            start=True,
                            stop=True,
                        )
                vd = sb1.tile([128, 4 * DIM], bf, tag="vd")
                nc.scalar.copy(vd[:], v_ps[:])

                if stage < 3:
                    nc.vector.tensor_copy(
                        y_g[:, 2 * s : 2 * s + 2, :].rearrange("p a b -> p (a b)"),
                        qk[:, 0:SC_ROWS],
                    )
                    continue

                # ---- attn^T: k q^T per (window, head) ----
                # ps_at [128, 1024] f32: bank b=h//2 (cols 512b); head parity hp=h%2:
                #   rows 64hp, col 512b + 64w + n;  tile_position (32h, 64hp)
                at_ps = ps_at.tile([128, 1024], f32, tag="at_ps")
                for w in range(SC_W):
                    for h in range(H):
                        hp, hb = h % 2, h // 2
                        nc.tensor.matmul(
                            at_ps[64 * hp : 64 * hp + 64, 512 * hb + 64 * w : 512 * hb + 64 * (w + 1)],
                            qk[32 * h : 32 * h + 32, SC_ROWS + 64 * w : SC_ROWS + 64 * (w + 1)],
                            qk[32 * h : 32 * h + 32, 64 * w : 64 * (w + 1)],
                            tile_position=(32 * h, 64 * hp),
                            start=True,
                            stop=True,
                        )

                # ---- softmax (no max subtraction; logits are O(1)) ----
                # pt/pb/pn [128, 512] bf16: col = 256*(h//2) + 64w + n; row 64*(h%2)+m
                pt = sb2.tile([128, 2 * SC_ROWS], bf, tag="pt")
                nc.scalar.activation(
                    pt[:].rearrange("p (b c) -> p b c", b=2),
                    at_ps[:].rearrange("p (b c) -> p b c", b=2)[:, :, 0:SC_ROWS],
                    EXP,
                )

                if stage < 4:
                    nc.vector.tensor_copy(
                        y_g[:, 2 * s : 2 * s + 2, :].rearrange("p a b -> p (a b)"),
                        pt[:, 0:SC_ROWS],
                    )
                    continue

                pb = sb2.tile([128, 2 * SC_ROWS], bf, tag="pb")
                nc.vector.tensor_mul(pb[:], pt[:], eb[:, 0 : 2 * SC_ROWS])

                # ---- softmax sums + reciprocal ----
                sm_ps = ps_sm.tile([128, 2 * SC_ROWS], f32, tag="sm_ps")
                for b_ in range(2):
                    for p_ in range(2):
                        nc.tensor.matmul(
                            sm_ps[64 * p_ : 64 * p_ + 64, 256 * b_ : 256 * b_ + SC_ROWS],
                            ones64[64 * p_ : 64 * p_ + 64, :],
                            pb[64 * p_ : 64 * p_ + 64, 256 * b_ : 256 * b_ + SC_ROWS],
                            tile_position=(64 * p_, 64 * p_),
                            start=True,
                            stop=True,
                        )
                rf = sb2.tile([128, 2 * SC_ROWS], f32, tag="rf")
                nc.vector.reciprocal_approx_fast(out=rf[:], in_=sm_ps[:])
                pn = sb2.tile([128, 2 * SC_ROWS], bf, tag="pn")
                nc.vector.tensor_mul(pn[:], pb[:], rf[:])

                if stage < 5:
                    nc.vector.tensor_copy(
                        y_g[:, 2 * s : 2 * s + 2, :].rearrange("p a b -> p (a b)"),
                        pn[:, 0:SC_ROWS],
                    )
                    continue

                # ---- av^T[(h,d), n] ----
                # ps_av [128, 256] f32: out [32h:32h+32, 64w:64w+64]; tile_position (64*(h%2), 32h)
                av_ps = ps_av.tile([128, SC_ROWS], f32, tag="av_ps")
                for w in range(SC_W):
                    for h in range(H):
                        hp, hb = h % 2, h // 2
                        nc.tensor.matmul(
                            av_ps[32 * h : 32 * h + 32, 64 * w : 64 * (w + 1)],
                            vd[64 * hp : 64 * hp + 64, 128 * w + 32 * h : 128 * w + 32 * (h + 1)],
                            pn[64 * hp : 64 * hp + 64, 256 * hb + 64 * w : 256 * hb + 64 * (w + 1)],
                            tile_position=(64 * hp, 32 * h),
                            start=True,
                            stop=True,
                        )
                av = sb1.tile([128, SC_ROWS], bf, tag="av")
                nc.scalar.copy(av[:], av_ps[:])

                if stage < 6:
                    nc.vector.tensor_copy(
                        y_g[:, 2 * s : 2 * s + 2, :].rearrange("p a b -> p (a b)"),
                        av[:],
                    )
                    continue

                # ---- y = av @ w_proj (+ b fused in evacuation) ----
                y_ps = ps_y.tile([128, 2 * DIM], f32, tag="y_ps")
                for j in range(2):
                    nc.tensor.matmul(
                        y_ps[:, 128 * j : 128 * (j + 1)],
                        av[:, 128 * j : 128 * (j + 1)],
                        wp[:],
                        start=True,
                        stop=True,
                    )
                nc.vector.tensor_add(
                    y_g[:, 2 * s : 2 * s + 2, :].rearrange("p a b -> p (a b)"),
                    y_ps[:],
                    br[:],
                )

            nc.sync.dma_start(
                out=y_d[g_r0 : g_r0 + GROUP * SC_ROWS, :].rearrange(
                    "(t p) c -> p t c", p=128
                ),
                in_=y_g[:],
            )

    if compile:
        nc.compile()
    _BUILT[key] = nc
    return nc


def _host_prep(w_qkv, w_proj, b_proj, bias_table):
    """Precompute replicated small tensors."""
    scale = HD**-0.5
    wq = (w_qkv[:, :DIM] * scale).astype(bf16)
    wk = w_qkv[:, DIM : 2 * DIM].astype(bf16)
    wv = w_qkv[:, 2 * DIM :].astype(bf16)
    wp = w_proj.astype(bf16)

    rel = _rel_pos_index(WS)  # [N, N]
    rel_bias = bias_table[rel.reshape(-1)].reshape(N, N, H).transpose(2, 0, 1)  # [h,n,m]
    ebv = np.exp(rel_bias).astype(np.float32)  # [h, n, m]
    # eb[p, 256*b + 64*w + n] = ebv[2b + (p>=64), n, p%64]
    eb = np.zeros((DIM, 2 * H * N // 1), np.float32)[:, : 2 * H * N]
    eb = np.zeros((DIM, 512), np.float32)
    for b_ in range(2):
        for p_ in range(2):
            h = 2 * b_ + p_
            blk = ebv[h].T  # [m, n]
            for w in range(SC_W):
                eb[64 * p_ : 64 * p_ + 64, 256 * b_ + 64 * w : 256 * b_ + 64 * (w + 1)] = blk
    eb = eb.astype(bf16)

    brep = np.tile(b_proj.astype(np.float32)[None, :], (DIM, 2))
    return wq, wk, wv, wp, eb, brep


def run(x, w_qkv, w_proj, b_proj, bias_table, trace=False, **trace_kwargs):
    """Run on 8 NeuronCores. Returns (y, BassKernelResults)."""
    from concourse import bass_utils

    x = np.asarray(x, dtype=np.float32)
    w_qkv = np.asarray(w_qkv, dtype=np.float32)
    w_proj = np.asarray(w_proj, dtype=np.float32)
    b_proj = np.asarray(b_proj, dtype=np.float32)
    bias_table = np.asarray(bias_table, dtype=np.float32)

    wq, wk, wv, wp, eb, brep = _host_prep(w_qkv, w_proj, b_proj, bias_table)
    nc = _build_program()

    xs = x.reshape(B * N, DIM)
    in_maps = []
    for c in range(NCORES):
        in_maps.append(
            {
                "x": np.ascontiguousarray(xs[c * ROWS : (c + 1) * ROWS]),
                "wq_bf": wq,
                "wk_bf": wk,
                "wv_bf": wv,
                "wp_bf": wp,
                "expbias_bf": eb,
                "b_rep": brep,
            }
        )

    res = bass_utils.run_bass_kernel_spmd(
        nc, in_maps, core_ids=list(range(NCORES)), trace=trace, **trace_kwargs
    )
    y = np.concatenate([res.results[c]["y"] for c in range(NCORES)], axis=0)
    return y.reshape(B, N, DIM), res


def kernel(x, w_qkv, w_proj, b_proj, bias_table):
    y, _ = run(x, w_qkv, w_proj, b_proj, bias_table)
    return y


if __name__ == "__main__":
    sys.path.insert(0, "/root/problem")
    import reference

    inputs = {k: np.asarray(v) for k, v in reference.setup_inputs().items()}
    out = kernel(**inputs)
    exp = np.asarray(reference.reference(**inputs))
    err = np.abs(out - exp)
    print("abs max err:", err.max(), "scale-rel:", err.max() / np.abs(exp).max())



# revision 2
# speedup vs baseline: 1.8082x; 1.8082x over previous
"""Trainium2 Bass kernel for windowed multi-head attention (Swin-style), v7.

v6 dataflow + software-pipelined (skewed) emission so no engine head-blocks.

Per-superchunk stages (sc = 4 windows, 256 tokens):
  pre(i):   T-mm x2 (PE), xt evac (DVE), QK-mm x2 (PE), qk evac (ACT),
            V-mm x2 (PE), v evac (DVE), cross-dup (DVE half, Pool half)
  mid(i):   attn x16 (PE), exp (ACT), pb = pt*eb (Pool)
  post1(i): rowsum-mm x2 (PE), recip (DVE), av x16 (PE), avn = av*rf (DVE)
  post2(i): proj x2 (PE), y evac (ACT)

Beat t emits: [dma], pre(t), mid(t-1), post1(t-2), post2(t-3).
PE order inside a beat: attn(t-1), T(t), QK(t), V(t), SM(t-2), AV(t-2), proj(t-3)
which matches steady-state readiness, so queues drain without stalls.

b_proj is added on the host (zeros in this problem).
"""

import sys

sys.path.insert(0, "/opt/trn_rl_repo")

import numpy as np
import ml_dtypes

WS = 8
H = 4
DIM = 128
N = WS * WS
HD = DIM // H
B = 4096
NCORES = 8
BC = B // NCORES
ROWS = BC * N

SC_W = 4
SC_ROWS = SC_W * N  # 256
N_SC = BC // SC_W  # 128
GROUP = 4
N_G = N_SC // GROUP  # 32

bf16 = ml_dtypes.bfloat16


def _rel_pos_index(ws: int) -> np.ndarray:
    coords = np.stack(np.meshgrid(np.arange(ws), np.arange(ws), indexing="ij"))
    flat = coords.reshape(2, -1)
    rel = flat[:, :, None] - flat[:, None, :]
    rel = rel.transpose(1, 2, 0).astype(np.int64)
    rel[..., 0] += ws - 1
    rel[..., 1] += ws - 1
    rel[..., 0] *= 2 * ws - 1
    return rel.sum(-1)


_BUILT = {}


def _build_program(n_groups=N_G, compile=True):
    key = (n_groups, compile)
    if key in _BUILT:
        return _BUILT[key]
    n_sc = n_groups * GROUP
    rows = n_sc * SC_ROWS

    from contextlib import ExitStack

    import concourse.tile as tile
    from concourse import bacc, mybir
    from concourse.masks import make_identity

    f32 = mybir.dt.float32
    bf = mybir.dt.bfloat16
    EXP = mybir.ActivationFunctionType.Exp

    nc = bacc.Bacc("TRN2", target_bir_lowering=False, debug=False, enable_asserts=False)

    x_d = nc.dram_tensor("x", [rows, DIM], f32, kind="ExternalInput").ap()
    wq_d = nc.dram_tensor("wq_bf", [DIM, DIM], bf, kind="ExternalInput").ap()
    wk_d = nc.dram_tensor("wk_bf", [DIM, DIM], bf, kind="ExternalInput").ap()
    wv_d = nc.dram_tensor("wv_bf", [DIM, DIM], bf, kind="ExternalInput").ap()
    wp_d = nc.dram_tensor("wp_bf", [DIM, DIM], bf, kind="ExternalInput").ap()
    eb_d = nc.dram_tensor("expbias_bf", [DIM, 2 * H * N], bf, kind="ExternalInput").ap()
    y_d = nc.dram_tensor("y", [rows, DIM], f32, kind="ExternalOutput").ap()

    with tile.TileContext(nc) as tc, ExitStack() as ctx:
        consts = ctx.enter_context(tc.tile_pool(name="consts", bufs=1))
        xp = ctx.enter_context(tc.tile_pool(name="xp", bufs=2))
        sb1 = ctx.enter_context(tc.tile_pool(name="sb1", bufs=4))
        sb2 = ctx.enter_context(tc.tile_pool(name="sb2", bufs=3))
        yp = ctx.enter_context(tc.tile_pool(name="yp", bufs=2))

        ps_t = ctx.enter_context(tc.tile_pool(name="ps_t", bufs=1, space="PSUM"))
        ps_qk = ctx.enter_context(tc.tile_pool(name="ps_qk", bufs=1, space="PSUM"))
        ps_v = ctx.enter_context(tc.tile_pool(name="ps_v", bufs=1, space="PSUM"))
        ps_at = ctx.enter_context(tc.tile_pool(name="ps_at", bufs=1, space="PSUM"))
        ps_r = ctx.enter_context(tc.tile_pool(name="ps_r", bufs=1, space="PSUM"))
        ps_av = ctx.enter_context(tc.tile_pool(name="ps_av", bufs=1, space="PSUM"))
        ps_y = ctx.enter_context(tc.tile_pool(name="ps_y", bufs=1, space="PSUM"))

        wq = consts.tile([DIM, DIM], bf)
        wk = consts.tile([DIM, DIM], bf)
        wv = consts.tile([DIM, DIM], bf)
        wp = consts.tile([DIM, DIM], bf)
        eb = consts.tile([DIM, 2 * H * N], bf)
        nc.sync.dma_start(wq[:], wq_d)
        nc.sync.dma_start(wk[:], wk_d)
        nc.sync.dma_start(wv[:], wv_d)
        nc.sync.dma_start(wp[:], wp_d)
        nc.sync.dma_start(eb[:], eb_d)

        ident = consts.tile([DIM, DIM], f32)
        make_identity(nc, ident[:])
        onesX = consts.tile([DIM, N], bf)
        nc.vector.memset(onesX[:], 0.0)
        nc.vector.memset(onesX[0:64, 0:32], 1.0)
        nc.vector.memset(onesX[64:128, 32:64], 1.0)

        # live tiles keyed by sc index (rotating via pools)
        live = {}

        def dma_in(g):
            x_g = xp.tile([128, 2 * GROUP, DIM], f32, tag="x_g")
            live[("x", g)] = x_g
            g_r0 = g * GROUP * SC_ROWS
            nc.sync.dma_start(
                out=x_g[:],
                in_=x_d[g_r0 : g_r0 + GROUP * SC_ROWS, :].rearrange(
                    "(t p) c -> p t c", p=128
                ),
            )

        def pre_PE_T(i):
            x_g = live[("x", i // GROUP)]
            s = i % GROUP
            t_ps = ps_t.tile([128, 2, DIM], f32, tag="t_ps")
            live[("t_ps", i)] = t_ps
            for t in range(2):
                nc.tensor.matmul(
                    t_ps[:, t, :],
                    x_g[:, 2 * s + t, :],
                    ident[:],
                    is_transpose=True,
                    start=True,
                    stop=True,
                )

        def pre_DVE_xt(i):
            t_ps = live.pop(("t_ps", i))
            xt = sb1.tile([128, SC_ROWS], bf, tag="xt")
            live[("xt", i)] = xt
            nc.vector.tensor_copy(xt[:].rearrange("p (a b) -> p a b", a=2), t_ps[:])

        def pre_PE_QK(i):
            xt = live[("xt", i)]
            qk_ps = ps_qk.tile([128, 2 * SC_ROWS], f32, tag="qk_ps")
            live[("qk_ps", i)] = qk_ps
            nc.tensor.matmul(qk_ps[:, 0:SC_ROWS], wq[:], xt[:], start=True, stop=True)
            nc.tensor.matmul(
                qk_ps[:, SC_ROWS : 2 * SC_ROWS], wk[:], xt[:], start=True, stop=True
            )

        def pre_ACT_qkE(i):
            qk_ps = live.pop(("qk_ps", i))
            qk = sb1.tile([128, 2 * SC_ROWS], bf, tag="qk")
            live[("qk", i)] = qk
            nc.scalar.copy(qk[:], qk_ps[:])

        def pre_PE_V(i):
            xt = live.pop(("xt", i))
            v_ps = ps_v.tile([128, 2, DIM], f32, tag="v_ps")
            live[("v_ps", i)] = v_ps
            for g2 in range(2):
                nc.tensor.matmul(
                    v_ps[:, g2, :],
                    xt[:, 128 * g2 : 128 * (g2 + 1)],
                    wv[:],
                    start=True,
                    stop=True,
                )

        def pre_DVE_vE(i):
            v_ps = live.pop(("v_ps", i))
            vsb = sb1.tile([128, 2, DIM], bf, tag="vsb")
            live[("vsb", i)] = vsb
            nc.vector.tensor_copy(vsb[:], v_ps[:])

        def pre_Pool_dup(i):
            vsb = live[("vsb", i)]
            vd2 = sb1.tile([128, 2, DIM], bf, tag="vd2")
            live[("vd2", i)] = vd2
            nc.gpsimd.tensor_copy(vd2[0:64, :, :], vsb[64:128, :, :])
            nc.gpsimd.tensor_copy(vd2[64:128, :, :], vsb[0:64, :, :])

        def mid_PE_attn(i):
            qk = live.pop(("qk", i))
            at_ps = ps_at.tile([128, 1024], f32, tag="at_ps")
            live[("at_ps", i)] = at_ps
            for w in range(SC_W):
                for h in range(H):
                    hp, hb = h % 2, h // 2
                    nc.tensor.matmul(
                        at_ps[
                            64 * hp : 64 * hp + 64,
                            512 * hb + 64 * w : 512 * hb + 64 * (w + 1),
                        ],
                        qk[32 * h : 32 * h + 32, SC_ROWS + 64 * w : SC_ROWS + 64 * (w + 1)],
                        qk[32 * h : 32 * h + 32, 64 * w : 64 * (w + 1)],
                        tile_position=(32 * h, 64 * hp),
                        start=True,
                        stop=True,
                    )

        def mid_ACT_exp(i):
            at_ps = live.pop(("at_ps", i))
            pt = sb2.tile([128, 2 * SC_ROWS], bf, tag="pt")
            live[("pt", i)] = pt
            nc.scalar.activation(
                pt[:].rearrange("p (b c) -> p b c", b=2),
                at_ps[:].rearrange("p (b c) -> p b c", b=2)[:, :, 0:SC_ROWS],
                EXP,
            )

        def mid_DVE_eb0(i):
            pt = live[("pt", i)]
            pb = sb2.tile([128, 2 * SC_ROWS], bf, tag="pb")
            live[("pb", i)] = pb
            nc.vector.tensor_mul(pb[:, 0:256], pt[:, 0:256], eb[:, 0:256])

        def mid_Pool_eb1(i):
            pt = live.pop(("pt", i))
            pb = live[("pb", i)]
            nc.gpsimd.tensor_mul(pb[:, 256:512], pt[:, 256:512], eb[:, 256:512])

        def post1_PE_SM(i):
            pb = live[("pb", i)]
            r_ps = ps_r.tile([128, SC_ROWS], f32, tag="r_ps")
            live[("r_ps", i)] = r_ps
            for hb in range(2):
                nc.tensor.matmul(
                    r_ps[64 * hb : 64 * hb + 64, :],
                    onesX[:, 0:64],
                    pb[:, 256 * hb : 256 * hb + 256],
                    tile_position=(0, 64 * hb),
                    start=True,
                    stop=True,
                )

        def post1_DVE_recip(i):
            r_ps = live.pop(("r_ps", i))
            rf = sb2.tile([128, SC_ROWS], mybir.dt.float32, tag="rf")
            live[("rf", i)] = rf
            nc.vector.reciprocal_approx_fast(out=rf[:], in_=r_ps[:])

        def post1_PE_AV(i):
            pb = live.pop(("pb", i))
            vsb = live.pop(("vsb", i))
            vd2 = live.pop(("vd2", i))
            av_ps = ps_av.tile([128, SC_ROWS], f32, tag="av_ps")
            live[("av_ps", i)] = av_ps
            for w in range(SC_W):
                for h in range(H):
                    hp, hb = h % 2, h // 2
                    src = vsb if (w % 2) == hp else vd2
                    nc.tensor.matmul(
                        av_ps[32 * h : 32 * h + 32, 64 * w : 64 * (w + 1)],
                        src[64 * hp : 64 * hp + 64, w // 2, 32 * h : 32 * (h + 1)],
                        pb[64 * hp : 64 * hp + 64, 256 * hb + 64 * w : 256 * hb + 64 * (w + 1)],
                        tile_position=(64 * hp, 32 * h),
                        start=True,
                        stop=True,
                    )

        def post1_DVE_avn(i):
            av_ps = live.pop(("av_ps", i))
            rf = live.pop(("rf", i))
            avn = sb1.tile([128, SC_ROWS], bf, tag="avn")
            live[("avn", i)] = avn
            nc.vector.tensor_mul(avn[:], av_ps[:], rf[:])

        def post2_PE_proj(i):
            avn = live.pop(("avn", i))
            y_ps = ps_y.tile([128, 2, DIM], f32, tag="y_ps")
            live[("y_ps", i)] = y_ps
            for j in range(2):
                nc.tensor.matmul(
                    y_ps[:, j, :],
                    avn[:, 128 * j : 128 * (j + 1)],
                    wp[:],
                    start=True,
                    stop=True,
                )

        def post2_ACT_yE(i):
            y_ps = live.pop(("y_ps", i))
            s = i % GROUP
            if s == 0:
                y_g = yp.tile([128, 2 * GROUP, DIM], f32, tag="y_g")
                live[("y", i // GROUP)] = y_g
            y_g = live[("y", i // GROUP)]
            nc.scalar.copy(y_g[:, 2 * s : 2 * s + 2, :], y_ps[:])
            if s == GROUP - 1:
                g = i // GROUP
                y_g = live.pop(("y", g))
                g_r0 = g * GROUP * SC_ROWS
                nc.sync.dma_start(
                    out=y_d[g_r0 : g_r0 + GROUP * SC_ROWS, :].rearrange(
                        "(t p) c -> p t c", p=128
                    ),
                    in_=y_g[:],
                )
                live.pop(("x", g), None)

        # ---- skewed emission ----
        dma_in(0)
        for t in range(n_sc + 4):
            # prefetch next x group two beats early
            if t + 2 < n_sc and (t + 2) % GROUP == 0:
                dma_in((t + 2) // GROUP)
            # pre-stages run two beats ahead of mid so qkE(t) never sits on
            # the attn->exp critical cycle
            if t < n_sc:
                pre_PE_T(t)
                pre_DVE_xt(t)
                pre_PE_QK(t)
            if 2 <= t <= n_sc + 1:
                mid_PE_attn(t - 2)
                mid_ACT_exp(t - 2)
            if t < n_sc:
                pre_ACT_qkE(t)
                pre_PE_V(t)
                pre_DVE_vE(t)
            if 2 <= t <= n_sc + 1:
                mid_DVE_eb0(t - 2)
                mid_Pool_eb1(t - 2)
            if t < n_sc:
                pre_Pool_dup(t)
            if 3 <= t <= n_sc + 2:
                post1_PE_SM(t - 3)
                post1_DVE_recip(t - 3)
                post1_PE_AV(t - 3)
                post1_DVE_avn(t - 3)
            if 4 <= t:
                post2_PE_proj(t - 4)
                post2_ACT_yE(t - 4)

    if compile:
        nc.compile()
    _BUILT[key] = nc
    return nc


def _host_prep(w_qkv, w_proj, b_proj, bias_table):
    scale = HD**-0.5
    wq = (w_qkv[:, :DIM] * scale).astype(bf16)
    wk = w_qkv[:, DIM : 2 * DIM].astype(bf16)
    wv = w_qkv[:, 2 * DIM :].astype(bf16)
    wp = w_proj.astype(bf16)

    rel = _rel_pos_index(WS)
    rel_bias = bias_table[rel.reshape(-1)].reshape(N, N, H).transpose(2, 0, 1)
    ebv = np.exp(rel_bias).astype(np.float32)
    eb = np.zeros((DIM, 512), np.float32)
    for hb in range(2):
        for hp in range(2):
            h = 2 * hb + hp
            blk = ebv[h].T
            for w in range(SC_W):
                eb[64 * hp : 64 * hp + 64, 256 * hb + 64 * w : 256 * hb + 64 * (w + 1)] = blk
    eb = eb.astype(bf16)
    return wq, wk, wv, wp, eb


def run(x, w_qkv, w_proj, b_proj, bias_table, trace=False, **trace_kwargs):
    from concourse import bass_utils

    x = np.asarray(x, dtype=np.float32)
    w_qkv = np.asarray(w_qkv, dtype=np.float32)
    w_proj = np.asarray(w_proj, dtype=np.float32)
    b_proj = np.asarray(b_proj, dtype=np.float32)
    bias_table = np.asarray(bias_table, dtype=np.float32)

    wq, wk, wv, wp, eb = _host_prep(w_qkv, w_proj, b_proj, bias_table)
    nc = _build_program()

    xs = x.reshape(B * N, DIM)
    in_maps = []
    for c in range(NCORES):
        in_maps.append(
            {
                "x": np.ascontiguousarray(xs[c * ROWS : (c + 1) * ROWS]),
                "wq_bf": wq,
                "wk_bf": wk,
                "wv_bf": wv,
                "wp_bf": wp,
                "expbias_bf": eb,
            }
        )

    res = bass_utils.run_bass_kernel_spmd(
        nc, in_maps, core_ids=list(range(NCORES)), trace=trace, **trace_kwargs
    )
    y = np.concatenate([res.results[c]["y"] for c in range(NCORES)], axis=0)
    y = y.reshape(B, N, DIM) + b_proj[None, None, :]
    return y, res


def kernel(x, w_qkv, w_proj, b_proj, bias_table):
    y, _ = run(x, w_qkv, w_proj, b_proj, bias_table)
    return y


if __name__ == "__main__":
    sys.path.insert(0, "/root/problem")
    import reference

    inputs = {k: np.asarray(v) for k, v in reference.setup_inputs().items()}
    out = kernel(**inputs)
    exp = np.asarray(reference.reference(**inputs))
    err = np.abs(out - exp)
    print("abs max err:", err.max(), "scale-rel:", err.max() / np.abs(exp).max())
